# revision 6
# baseline (speedup 1.0000x reference)
"""BiologicalSplatAttentionLayer Trainium2 kernel (8-core SPMD).

Math (per batch b):
    aff[s,k]  = normalize_k( exp(-max(|x_s - c_k|^2, 0) / (2 sig_k^2)) )
    out       = aff @ ((aff.T @ x) @ Wv.T @ Wo.T)
The factored form is algebraically identical to the reference
(values/splat_states associativity through the rank-K bottleneck) and turns
two SxDxD matmuls into KxDxD ones.

Sharding: 8 cores = 4 batches x 2 token-halves. y = aff.T @ x couples all
tokens of a batch; on-device collectives measure ~100us fixed here, so each
core redundantly processes its full batch for the affinity/aggregation phase
and computes only its own token-half of the output. Each core's token stream
is reordered (own half first) host-side so the SPMD program always outputs
chunks 0..15.

The squared-distance path runs in fp8-e4m3 (x pre-scaled by 8, centers by 16,
both scales folded into the per-splat affine constants) with DoubleRow
matmuls (256-deep contraction per pass). The fp8 error enters the distance
dot product and is averaged down by sqrt(D); the y/value path stays bf16.
All DRAM tensors are laid out partition-major so every DMA is a fully
coalesced [128, N] transfer; the host does the (free) permutes.

Host-side prep is data layout (slices, permutes, bf16/fp8 casts) plus
parameter preprocessing: the two projection weights are fused
(Wc = Wv.T @ Wo.T, exact fp32) and the 64 splat scale constants
(1/(2 sig^2), |c|^2) are folded, exactly as a deployed model would at load
time. All per-token arithmetic (affinities, normalization, aggregation
matmuls) runs on-device.
"""

import numpy as np
import ml_dtypes

import concourse.bass as bass
import concourse.tile as tile
import concourse.mybir as mybir
from concourse import bacc
from concourse import bass_utils

BF16 = mybir.dt.bfloat16
F32 = mybir.dt.float32
FP8 = mybir.dt.float8e4
NPBF16 = ml_dtypes.bfloat16
NPFP8 = mybir.dt.np(mybir.dt.float8e4)

B, S, D, K = 4, 4096, 1024, 64
NCORES = 8
SH = S // 2            # output tokens per core
NCH = S // 128         # processed 128-token chunks per core (32)
NOCH = SH // 128       # output chunks per core (16)
NB = S // 512          # processed 512-token blocks per core (8)
ND = D // 128          # contraction chunks (8)
NDP = ND // 2          # DoubleRow contraction chunk pairs (4)
SBS = [1, 2, 2, 3]     # superblock sizes (blocks sharing a weights-outer xc loop)
XS = 8.0               # fp8 pre-scale for the transposed x stream
CS = 16.0              # fp8 pre-scale for the splat centers

_CACHE = {}


def _build_nc():
    nc = bacc.Bacc("TRN2", debug=False, enable_asserts=False, num_devices=NCORES)

    xn_d = nc.dram_tensor("xn", [128, NB * 4 * D], BF16, kind="ExternalInput")
    xt_d = nc.dram_tensor("xt", [128, NB * NDP * 2 * 512], FP8, kind="ExternalInput")
    c8_d = nc.dram_tensor("c8", [128, NDP * 2 * K], FP8, kind="ExternalInput")
    cb_d = nc.dram_tensor("cb", [128, 128], BF16, kind="ExternalInput")   # identity
    cf_d = nc.dram_tensor("cf", [128, 66], F32, kind="ExternalInput")     # invb|c2s|invs
    wc_d = nc.dram_tensor("wc", [128, ND * D], BF16, kind="ExternalInput")
    out_d = nc.dram_tensor("out", [128, (NOCH // 2) * 2 * D], BF16,
                           kind="ExternalOutput")

    with tile.TileContext(nc) as tc:
        with (
            tc.tile_pool(name="const", bufs=1) as cpool,
            tc.tile_pool(name="xts", bufs=7) as xt_pool,
            tc.tile_pool(name="xns", bufs=7) as xn_pool,
            tc.tile_pool(name="adj", bufs=5) as adj_pool,
            tc.tile_pool(name="tsb", bufs=8) as t_pool,
            tc.tile_pool(name="osb", bufs=3) as o_pool,
        ):
            # ---- identity + fp8 centers + block-0 xt first on the sync FIFO so
            # the first xc matmuls have operands ASAP; f32 consts follow ----
            idb = cpool.tile([128, 128], BF16)
            nc.sync.dma_start(idb[:], cb_d.ap())
            c8_sb = cpool.tile([128, NDP, 2, K], FP8)
            nc.sync.dma_start(c8_sb[:], c8_d.ap())

            xt_ts = {}
            xn_ts = {}
            xt_ts[0] = xt_pool.tile([128, NDP, 2, 512], FP8, name="xt_t0", tag="xt_t")
            nc.sync.dma_start(xt_ts[0][:], xt_d.ap()[:, 0:NDP * 2 * 512])
            xn_ts[0] = xn_pool.tile([128, 4, D], BF16, name="xn_t0", tag="xn_t")
            nc.gpsimd.dma_start(xn_ts[0][:], xn_d.ap()[:, 0:4 * D])

            cf_sb = cpool.tile([128, 66], F32)
            nc.sync.dma_start(cf_sb[:], cf_d.ap())
            invb = cf_sb[:, 0:64]                 # inv broadcast tile
            c2s_col = cf_sb[0:64, 64:65]          # XS * CS * |c|^2
            invs_col = cf_sb[0:64, 65:66]         # inv / (XS * CS)

            wc_sb = cpool.tile([128, ND, D], BF16)

            x2_sb = cpool.tile([128, NCH], F32)      # |x_s|^2 per chunk column
            den_sb = cpool.tile([128, NCH], F32)
            rden_sb = cpool.tile([128, NCH], F32)
            afft_sb = cpool.tile([64, SH], BF16)     # aff.T for own-half chunks
            z_bf = cpool.tile([K, D], BF16)
            y_bf = cpool.tile([K, D], BF16)
            yt_sb = cpool.tile([128, ND, K], BF16)

            # ---- phase 1: affinities + y = aff.T @ x over the full batch ----------
            with (
                tc.tile_pool(name="psxc", bufs=3, space="PSUM") as psxc,
                tc.tile_pool(name="pstr", bufs=3, space="PSUM") as pstr,
                tc.tile_pool(name="psy", bufs=1, space="PSUM") as psy,
                tc.tile_pool(name="affp", bufs=8) as aff_pool,
            ):
                # nudge the PE clock gate while inputs stream in
                warm_ps = psxc.tile([K, 512], F32, name="warm", tag="xc")
                for w in range(12):
                    nc.tensor.matmul(
                        warm_ps[:, 0:128], idb[0:64, 0:64], idb[0:64, :],
                        start=True, stop=True,
                    )
                psum_y = psy.tile([K, D], F32)

                def emit_stream(blk):
                    if blk >= NB:
                        return
                    if blk > 0:
                        xt_t = xt_pool.tile(
                            [128, NDP, 2, 512], FP8, name=f"xt_t{blk}", tag="xt_t"
                        )
                        nc.sync.dma_start(
                            xt_t[:], xt_d.ap()[:, blk * 4096:(blk + 1) * 4096]
                        )
                        xn_t = xn_pool.tile(
                            [128, 4, D], BF16, name=f"xn_t{blk}", tag="xn_t"
                        )
                        nc.gpsimd.dma_start(
                            xn_t[:], xn_d.ap()[:, blk * 4 * D:(blk + 1) * 4 * D]
                        )
                        xt_ts[blk], xn_ts[blk] = xt_t, xn_t
                    if blk == 5:
                        # fused projection weight (phase 2 only), behind most
                        # of the xt stream on the sync FIFO
                        nc.sync.dma_start(wc_sb[:], wc_d.ap())

                def emit_xc(blk):
                    # DoubleRow fp8 distance matmul for one block
                    ps = psxc.tile([K, 512], F32, name=f"xc{blk}", tag="xc")
                    for djp in range(NDP):
                        nc.tensor.matmul(
                            ps[:],
                            c8_sb[:, djp, :, :],
                            xt_ts[blk][:, djp, :, :],
                            start=(djp == 0), stop=(djp == NDP - 1),
                            perf_mode=mybir.MatmulPerfMode.DoubleRow,
                        )
                    return ps

                def emit_chain(blk, psum_xc):
                    # affinity chain for one block: produces aff_bf tiles
                    xn_t = xn_ts[blk]
                    # adj = (XS*CS*2xc - XS*CS*c2) * (inv/(XS*CS))  (k-major)
                    adj_sb = adj_pool.tile([K, 512], BF16)
                    nc.vector.tensor_scalar(
                        adj_sb[:], psum_xc[:], c2s_col, invs_col,
                        mybir.AluOpType.subtract, mybir.AluOpType.mult,
                    )
                    t_blk = t_pool.tile([128, 4, 64], F32, tag="t")
                    affu_blk = t_pool.tile([128, 4, 64], F32, tag="affu")
                    for j2 in range(4):
                        j = blk * 4 + j2
                        if j2 == 3:
                            # one square per block runs on the DVE to level the
                            # scalar-engine load
                            sq_bf = t_pool.tile([128, D], BF16, tag="sqb")
                            nc.vector.tensor_mul(
                                sq_bf[:], xn_t[:, j2, :], xn_t[:, j2, :]
                            )
                            nc.vector.tensor_reduce(
                                x2_sb[:, j:j + 1], sq_bf[:],
                                mybir.AxisListType.X, mybir.AluOpType.add,
                            )
                        else:
                            sq = t_pool.tile([128, D], F32, tag="sq")
                            nc.scalar.activation(
                                sq[:], xn_t[:, j2, :],
                                mybir.ActivationFunctionType.Square,
                                accum_out=x2_sb[:, j:j + 1],
                            )
                        # transpose adj chunk -> [s, k]
                        bt_ps = pstr.tile([128, 64], BF16, tag="tr")
                        nc.tensor.transpose(
                            bt_ps[:], adj_sb[:, j2 * 128:(j2 + 1) * 128],
                            idb[0:64, 0:64],
                        )
                        # t = inv*x2 - inv*adj = inv * d2
                        # (reference clamps d2 at 0; d2<0 only arises from fp
                        # rounding and changes aff by <=1e-4 relative, so the
                        # clamp op is elided)
                        nc.vector.scalar_tensor_tensor(
                            t_blk[:, j2, :], invb, x2_sb[:, j:j + 1], bt_ps[:],
                            mybir.AluOpType.mult, mybir.AluOpType.subtract,
                        )
                    # one batched exp + free-dim sum per block
                    nc.scalar.activation(
                        affu_blk[:], t_blk[:], mybir.ActivationFunctionType.Exp,
                        scale=-1.0,
                    )
                    nc.vector.tensor_reduce(
                        den_sb[:, blk * 4:blk * 4 + 4],
                        affu_blk[:],
                        mybir.AxisListType.X, mybir.AluOpType.add,
                    )
                    # denominator guard + reciprocal batched per block, on the
                    # otherwise-idle gpsimd engine (SBUF-only operands)
                    nc.gpsimd.tensor_scalar_add(
                        den_sb[:, blk * 4:blk * 4 + 4],
                        den_sb[:, blk * 4:blk * 4 + 4], 1e-8,
                    )
                    nc.vector.reciprocal(
                        rden_sb[:, blk * 4:blk * 4 + 4],
                        den_sb[:, blk * 4:blk * 4 + 4],
                    )
                    affbs = []
                    for j2 in range(4):
                        j = blk * 4 + j2
                        aff_bf = aff_pool.tile([128, 64], BF16)
                        nc.gpsimd.tensor_scalar_mul(
                            aff_bf[:], affu_blk[:, j2, :], rden_sb[:, j:j + 1]
                        )
                        affbs.append(aff_bf)
                    return affbs

                def emit_y(blk, affbs):
                    # y matmuls for the whole block, back to back on the PE
                    for j2 in range(4):
                        j = blk * 4 + j2
                        for dh in range(2):
                            nc.tensor.matmul(
                                psum_y[:, dh * 512:(dh + 1) * 512],
                                affbs[j2][:],
                                xn_ts[blk][:, j2, dh * 512:(dh + 1) * 512],
                                start=(j == 0), stop=(j == NCH - 1),
                            )
                    # aff.T for the output matmul (own half only)
                    if blk * 4 < NOCH:
                        for j2 in range(4):
                            j = blk * 4 + j2
                            at_ps = pstr.tile([64, 128], BF16, tag="tr")
                            nc.tensor.transpose(at_ps[:], affbs[j2][:], idb)
                            nc.vector.tensor_copy(
                                afft_sb[:, j * 128:(j + 1) * 128], at_ps[:]
                            )

                # one-block-staggered software pipeline: while block b's
                # affinity chain crosses DVE/ACT, the PE runs block b+1's
                # distance matmuls, so neither engine head-of-line blocks
                emit_stream(0)
                emit_stream(1)
                ps_prev = emit_xc(0)
                aff_prev = emit_chain(0, ps_prev)
                for blk in range(1, NB):
                    emit_stream(blk + 1)
                    ps_cur = emit_xc(blk)
                    emit_y(blk - 1, aff_prev)
                    aff_prev = emit_chain(blk, ps_cur)
                emit_y(NB - 1, aff_prev)
                nc.vector.tensor_copy(y_bf[:], psum_y[:])

            # ---- phases 2+3 in a fresh PSUM scope ---------------------------------
            with (
                tc.tile_pool(name="pstr2", bufs=2, space="PSUM") as pstr2,
                tc.tile_pool(name="pswz", bufs=2, space="PSUM") as pswz,
                tc.tile_pool(name="pso", bufs=2, space="PSUM") as pso,
            ):
                # ---- phase 2: Z = y @ (Wv.T Wo.T)  (weights pre-fused on host) ----
                for dj in range(ND):
                    tr = pstr2.tile([128, 64], BF16, tag="tr")
                    nc.tensor.transpose(
                        tr[:], y_bf[:, dj * 128:(dj + 1) * 128], idb[0:64, 0:64]
                    )
                    nc.vector.tensor_copy(yt_sb[:, dj, :], tr[:])
                    # keep the PE clock gate warm through the transpose stretch
                    warm2 = pstr2.tile([K, 128], F32, name=f"warm2_{dj}", tag="tr")
                    nc.tensor.matmul(
                        warm2[:], idb[0:64, 0:64], idb[0:64, :],
                        start=True, stop=True,
                    )
                ps_z = [
                    pswz.tile([K, 512], F32, name=f"z{fh}", tag="wz")
                    for fh in range(2)
                ]
                for dj in range(ND):
                    for fh in range(2):
                        nc.tensor.matmul(
                            ps_z[fh][:],
                            yt_sb[:, dj, :],
                            wc_sb[:, dj, fh * 512:(fh + 1) * 512],
                            start=(dj == 0), stop=(dj == ND - 1),
                        )
                for fh in range(2):
                    nc.vector.tensor_copy(
                        z_bf[:, fh * 512:(fh + 1) * 512], ps_z[fh][:]
                    )

                # ---- phase 3: out = aff @ Z (own token half) ----------------------
                for g in range(NOCH // 2):
                    o_sb = o_pool.tile([128, 2, D], BF16)
                    for j2 in range(2):
                        j = g * 2 + j2
                        psum_o = pso.tile([128, D], F32)
                        for fh in range(2):
                            nc.tensor.matmul(
                                psum_o[:, fh * 512:(fh + 1) * 512],
                                afft_sb[:, j * 128:(j + 1) * 128],
                                z_bf[:, fh * 512:(fh + 1) * 512],
                                start=True, stop=True,
                            )
                        # alternate the PSUM->SBUF cast between DVE and ACT so
                        # neither engine paces phase 3
                        if j2 == 0:
                            nc.vector.tensor_copy(o_sb[:, j2, :], psum_o[:])
                        else:
                            nc.scalar.activation(
                                o_sb[:, j2, :], psum_o[:],
                                mybir.ActivationFunctionType.Copy,
                            )
                    nc.sync.dma_start(
                        out_d.ap()[:, g * 2 * D:(g + 1) * 2 * D], o_sb[:]
                    )

    nc.compile()
    return nc


def _get_nc():
    if "nc" not in _CACHE:
        _CACHE["nc"] = _build_nc()
    return _CACHE["nc"]


def kernel(token_embeddings, splat_centers, splat_log_scales, Wv, Wo):
    x = np.asarray(token_embeddings, dtype=np.float32)
    centers = np.asarray(splat_centers, dtype=np.float32)
    log_scales = np.asarray(splat_log_scales, dtype=np.float32)
    Wv = np.asarray(Wv, dtype=np.float32)
    Wo = np.asarray(Wo, dtype=np.float32)

    nc = _get_nc()

    # parameter preprocessing (folded exactly as at model-load time)
    sig = np.clip(np.exp(log_scales), 0.1, 2.0).astype(np.float32)
    inv = (0.5 / (sig * sig)).astype(np.float32)            # 1/(2 sig^2)
    c2 = np.einsum("kd,kd->k", centers, centers).astype(np.float32)

    # constants: identity, fp8 centers (DoubleRow pair layout), f32 affine blob
    cb = np.eye(128, dtype=NPBF16)
    ctb = np.ascontiguousarray((2.0 * CS) * centers.T).astype(NPFP8)   # [D, K]
    # d = djp*256 + i*128 + p  ->  c8[p, djp, i, k]
    c8 = ctb.reshape(NDP, 2, 128, K).transpose(2, 0, 1, 3).reshape(128, -1)
    cf = np.zeros((128, 66), dtype=np.float32)
    cf[:, 0:64] = np.tile(inv.reshape(1, K), (128, 1))
    cf[0:64, 64] = (XS * CS) * c2
    cf[0:64, 65] = inv / (XS * CS)

    wc = (Wv.T.astype(np.float32) @ Wo.T.astype(np.float32)).astype(NPBF16)
    wcr = wc.reshape(ND, 128, D).transpose(1, 0, 2).reshape(128, -1)

    shared = {"cb": cb, "cf": cf, "c8": np.ascontiguousarray(c8),
              "wc": np.ascontiguousarray(wcr)}
    in_maps = []
    for b in range(B):
        xb_bf = x[b].astype(NPBF16)                              # [S, D]
        xbt_f8 = np.ascontiguousarray(x[b].T * XS).astype(NPFP8)  # [D, S]
        for h in range(2):
            own = slice(h * SH, (h + 1) * SH)
            oth = slice((1 - h) * SH, (2 - h) * SH)
            m = dict(shared)
            xn = np.concatenate([xb_bf[own], xb_bf[oth]], axis=0)     # [S, D]
            xt = np.concatenate([xbt_f8[:, own], xbt_f8[:, oth]], axis=1)  # [D, S]
            # xn[blk*512 + c*128 + p, d] -> [p, blk, c, d]
            m["xn"] = np.ascontiguousarray(
                xn.reshape(NB, 4, 128, D).transpose(2, 0, 1, 3).reshape(128, -1)
            )
            # xt[djp*256 + i*128 + p, blk*512 + s] -> [p, blk, djp, i, s]
            m["xt"] = np.ascontiguousarray(
                xt.reshape(NDP, 2, 128, NB, 512).transpose(2, 3, 0, 1, 4)
                .reshape(128, -1)
            )
            in_maps.append(m)

    res = bass_utils.run_bass_kernel_spmd(nc, in_maps, core_ids=list(range(NCORES)))

    out = np.empty((B, S, D), dtype=np.float32)
    for c in range(NCORES):
        b, h = divmod(c, 2)
        # out_lin [128, g, j2, d] -> tokens t = g*256 + j2*128 + p
        ol = res.results[c]["out"].reshape(128, NOCH // 2, 2, D)
        out[b, h * SH:(h + 1) * SH] = (
            ol.transpose(1, 2, 0, 3).reshape(SH, D).astype(np.float32)
        )
    return out


# revision 8
# speedup vs baseline: 1.1448x; 1.1448x over previous
"""BiologicalSplatAttentionLayer Trainium2 kernel (8-core SPMD).

Math (per batch b):
    aff[s,k]  = normalize_k( exp(-max(|x_s - c_k|^2, 0) / (2 sig_k^2)) )
    out       = aff @ ((aff.T @ x) @ Wv.T @ Wo.T)
The factored form is algebraically identical to the reference
(values/splat_states associativity through the rank-K bottleneck) and turns
two SxDxD matmuls into KxDxD ones.

Sharding: 8 cores = 4 batches x 2 token-halves. y = aff.T @ x couples all
tokens of a batch; on-device collectives measure ~100us fixed here, so each
core redundantly processes its full batch for the affinity/aggregation phase
and computes only its own token-half of the output. Each core's token stream
is reordered (own half first) host-side so the SPMD program always outputs
chunks 0..15.

The squared-distance path runs in fp8-e4m3 (x pre-scaled by 8, centers by 16,
both scales folded into the per-splat affine constants) with DoubleRow
matmuls (256-deep contraction per pass). The fp8 error enters the distance
dot product and is averaged down by sqrt(D); the y/value path stays bf16.
All DRAM tensors are laid out partition-major so every DMA is a fully
coalesced [128, N] transfer; the host does the (free) permutes.

Host-side prep is data layout (slices, permutes, bf16/fp8 casts) plus
parameter preprocessing: the two projection weights are fused
(Wc = Wv.T @ Wo.T, exact fp32) and the 64 splat scale constants
(1/(2 sig^2), |c|^2) are folded, exactly as a deployed model would at load
time. All per-token arithmetic (affinities, normalization, aggregation
matmuls) runs on-device.
"""

import numpy as np
import ml_dtypes

import concourse.bass as bass
import concourse.tile as tile
import concourse.mybir as mybir
from concourse import bacc
from concourse import bass_utils

BF16 = mybir.dt.bfloat16
F32 = mybir.dt.float32
FP8 = mybir.dt.float8e4
NPBF16 = ml_dtypes.bfloat16
NPFP8 = mybir.dt.np(mybir.dt.float8e4)

B, S, D, K = 4, 4096, 1024, 64
NCORES = 8
SH = S // 2            # output tokens per core
NCH = S // 128         # processed 128-token chunks per core (32)
NOCH = SH // 128       # output chunks per core (16)
NB = S // 512          # processed 512-token blocks per core (8)
ND = D // 128          # contraction chunks (8)
NDP = ND // 2          # DoubleRow contraction chunk pairs (4)
SBS = [1, 2, 2, 3]     # superblock sizes (blocks sharing a weights-outer xc loop)
XS = 8.0               # fp8 pre-scale for the transposed x stream
CS = 16.0              # fp8 pre-scale for the splat centers

_CACHE = {}


def _build_nc():
    nc = bacc.Bacc("TRN2", debug=False, enable_asserts=False, num_devices=NCORES)

    xn_d = nc.dram_tensor("xn", [128, NB * 4 * D], BF16, kind="ExternalInput")
    xt_d = nc.dram_tensor("xt", [128, NB * NDP * 2 * 512], FP8, kind="ExternalInput")
    c8_d = nc.dram_tensor("c8", [128, NDP * 2 * K], FP8, kind="ExternalInput")
    cb_d = nc.dram_tensor("cb", [128, 128], BF16, kind="ExternalInput")   # identity
    cf_d = nc.dram_tensor("cf", [128, 66], F32, kind="ExternalInput")     # invb|c2s|invs
    wc_d = nc.dram_tensor("wc", [128, ND * D], BF16, kind="ExternalInput")
    out_d = nc.dram_tensor("out", [128, (NOCH // 2) * 2 * D], BF16,
                           kind="ExternalOutput")

    with tile.TileContext(nc) as tc:
        with (
            tc.tile_pool(name="const", bufs=1) as cpool,
            tc.tile_pool(name="xts", bufs=7) as xt_pool,
            tc.tile_pool(name="xns", bufs=7) as xn_pool,
            tc.tile_pool(name="adj", bufs=5) as adj_pool,
            tc.tile_pool(name="tsb", bufs=8) as t_pool,
            tc.tile_pool(name="osb", bufs=4) as o_pool,
        ):
            # ---- identity + fp8 centers + block-0 xt first on the sync FIFO so
            # the first xc matmuls have operands ASAP; f32 consts follow ----
            idb = cpool.tile([128, 128], BF16)
            nc.sync.dma_start(idb[:], cb_d.ap())
            c8_sb = cpool.tile([128, NDP, 2, K], FP8)
            nc.sync.dma_start(c8_sb[:], c8_d.ap())

            xt_ts = {}
            xn_ts = {}
            xt_ts[0] = xt_pool.tile([128, NDP, 2, 512], FP8, name="xt_t0", tag="xt_t")
            nc.sync.dma_start(xt_ts[0][:], xt_d.ap()[:, 0:NDP * 2 * 512])
            xn_ts[0] = xn_pool.tile([128, 4, D], BF16, name="xn_t0", tag="xn_t")
            nc.gpsimd.dma_start(xn_ts[0][:], xn_d.ap()[:, 0:4 * D])

            cf_sb = cpool.tile([128, 66], F32)
            nc.sync.dma_start(cf_sb[:], cf_d.ap())
            invb = cf_sb[:, 0:64]                 # inv broadcast tile
            c2s_col = cf_sb[0:64, 64:65]          # XS * CS * |c|^2
            invs_col = cf_sb[0:64, 65:66]         # inv / (XS * CS)

            wc_sb = cpool.tile([128, ND, D], BF16)

            x2_sb = cpool.tile([128, NCH], F32)      # |x_s|^2 per chunk column
            den_sb = cpool.tile([128, NCH], F32)
            rden_sb = cpool.tile([128, NCH], F32)
            afft_sb = cpool.tile([64, SH], BF16)     # aff.T for own-half chunks
            z_bf = cpool.tile([K, D], BF16)
            y_bf = cpool.tile([K, D], BF16)
            yt_sb = cpool.tile([128, ND, K], BF16)

            # ---- phase 1: affinities + y = aff.T @ x over the full batch ----------
            with (
                tc.tile_pool(name="psxc", bufs=3, space="PSUM") as psxc,
                tc.tile_pool(name="pstr", bufs=3, space="PSUM") as pstr,
                tc.tile_pool(name="psy", bufs=1, space="PSUM") as psy,
                tc.tile_pool(name="affp", bufs=12) as aff_pool,
            ):
                # nudge the PE clock gate while inputs stream in
                warm_ps = psxc.tile([K, 512], F32, name="warm", tag="xc")
                for w in range(12):
                    nc.tensor.matmul(
                        warm_ps[:, 0:128], idb[0:64, 0:64], idb[0:64, :],
                        start=True, stop=True,
                    )
                psum_y = psy.tile([K, D], F32)

                def emit_stream(blk):
                    if blk >= NB:
                        return
                    if blk > 0:
                        xt_t = xt_pool.tile(
                            [128, NDP, 2, 512], FP8, name=f"xt_t{blk}", tag="xt_t"
                        )
                        nc.sync.dma_start(
                            xt_t[:], xt_d.ap()[:, blk * 4096:(blk + 1) * 4096]
                        )
                        xn_t = xn_pool.tile(
                            [128, 4, D], BF16, name=f"xn_t{blk}", tag="xn_t"
                        )
                        nc.gpsimd.dma_start(
                            xn_t[:], xn_d.ap()[:, blk * 4 * D:(blk + 1) * 4 * D]
                        )
                        xt_ts[blk], xn_ts[blk] = xt_t, xn_t
                    if blk == 5:
                        # fused projection weight (phase 2 only), behind most
                        # of the xt stream on the sync FIFO
                        nc.sync.dma_start(wc_sb[:], wc_d.ap())

                def emit_xc(blk):
                    # DoubleRow fp8 distance matmul for one block
                    ps = psxc.tile([K, 512], F32, name=f"xc{blk}", tag="xc")
                    for djp in range(NDP):
                        nc.tensor.matmul(
                            ps[:],
                            c8_sb[:, djp, :, :],
                            xt_ts[blk][:, djp, :, :],
                            start=(djp == 0), stop=(djp == NDP - 1),
                            perf_mode=mybir.MatmulPerfMode.DoubleRow,
                        )
                    return ps

                def emit_chain(blk, psum_xc):
                    # affinity chain for one block: produces aff_bf tiles
                    xn_t = xn_ts[blk]
                    # adj = (XS*CS*2xc - XS*CS*c2) * (inv/(XS*CS))  (k-major)
                    adj_sb = adj_pool.tile([K, 512], BF16)
                    nc.vector.tensor_scalar(
                        adj_sb[:], psum_xc[:], c2s_col, invs_col,
                        mybir.AluOpType.subtract, mybir.AluOpType.mult,
                    )
                    t_blk = t_pool.tile([128, 4, 64], F32, tag="t")
                    affu_blk = t_pool.tile([128, 4, 64], F32, tag="affu")
                    for j2 in range(4):
                        j = blk * 4 + j2
                        sq = t_pool.tile([128, D], F32, tag="sq")
                        nc.scalar.activation(
                            sq[:], xn_t[:, j2, :],
                            mybir.ActivationFunctionType.Square,
                            accum_out=x2_sb[:, j:j + 1],
                        )
                        # transpose adj chunk -> [s, k]
                        bt_ps = pstr.tile([128, 64], BF16, tag="tr")
                        nc.tensor.transpose(
                            bt_ps[:], adj_sb[:, j2 * 128:(j2 + 1) * 128],
                            idb[0:64, 0:64],
                        )
                        # t = inv*x2 - inv*adj = inv * d2
                        # (reference clamps d2 at 0; d2<0 only arises from fp
                        # rounding and changes aff by <=1e-4 relative, so the
                        # clamp op is elided)
                        nc.vector.scalar_tensor_tensor(
                            t_blk[:, j2, :], invb, x2_sb[:, j:j + 1], bt_ps[:],
                            mybir.AluOpType.mult, mybir.AluOpType.subtract,
                        )
                    # one batched exp + free-dim sum per block
                    nc.scalar.activation(
                        affu_blk[:], t_blk[:], mybir.ActivationFunctionType.Exp,
                        scale=-1.0,
                    )
                    nc.vector.tensor_reduce(
                        den_sb[:, blk * 4:blk * 4 + 4],
                        affu_blk[:],
                        mybir.AxisListType.X, mybir.AluOpType.add,
                    )
                    # denominator guard + reciprocal batched per block, on the
                    # otherwise-idle gpsimd engine (SBUF-only operands)
                    nc.vector.tensor_scalar_add(
                        den_sb[:, blk * 4:blk * 4 + 4],
                        den_sb[:, blk * 4:blk * 4 + 4], 1e-8,
                    )
                    nc.vector.reciprocal(
                        rden_sb[:, blk * 4:blk * 4 + 4],
                        den_sb[:, blk * 4:blk * 4 + 4],
                    )
                    affbs = []
                    for j2 in range(4):
                        j = blk * 4 + j2
                        aff_bf = aff_pool.tile([128, 64], BF16)
                        nc.vector.tensor_scalar_mul(
                            aff_bf[:], affu_blk[:, j2, :], rden_sb[:, j:j + 1]
                        )
                        affbs.append(aff_bf)
                    return affbs

                def emit_y(blk, affbs):
                    # y matmuls for the whole block, back to back on the PE
                    for j2 in range(4):
                        j = blk * 4 + j2
                        for dh in range(2):
                            nc.tensor.matmul(
                                psum_y[:, dh * 512:(dh + 1) * 512],
                                affbs[j2][:],
                                xn_ts[blk][:, j2, dh * 512:(dh + 1) * 512],
                                start=(j == 0), stop=(j == NCH - 1),
                            )
                    # aff.T for the output matmul (own half only)
                    if blk * 4 < NOCH:
                        for j2 in range(4):
                            j = blk * 4 + j2
                            at_ps = pstr.tile([64, 128], BF16, tag="tr")
                            nc.tensor.transpose(at_ps[:], affbs[j2][:], idb)
                            nc.vector.tensor_copy(
                                afft_sb[:, j * 128:(j + 1) * 128], at_ps[:]
                            )

                # two-block-staggered software pipeline: while block b's
                # affinity chain crosses DVE/ACT, the PE runs blocks b+1/b+2's
                # distance matmuls and block b-1's y, so no engine
                # head-of-line blocks on the cross-engine chain
                emit_stream(0)
                emit_stream(1)
                emit_stream(2)
                ps = {0: emit_xc(0), 1: emit_xc(1)}
                affs_done = {}
                for blk in range(NB):
                    affs_done[blk] = emit_chain(blk, ps[blk])
                    if blk + 2 < NB:
                        emit_stream(blk + 3)
                        ps[blk + 2] = emit_xc(blk + 2)
                    if blk >= 1:
                        emit_y(blk - 1, affs_done.pop(blk - 1))
                emit_y(NB - 1, affs_done.pop(NB - 1))
                nc.vector.tensor_copy(y_bf[:], psum_y[:])

            # ---- phases 2+3 in a fresh PSUM scope ---------------------------------
            with (
                tc.tile_pool(name="pstr2", bufs=2, space="PSUM") as pstr2,
                tc.tile_pool(name="pswz", bufs=2, space="PSUM") as pswz,
                tc.tile_pool(name="pso", bufs=2, space="PSUM") as pso,
            ):
                # ---- phase 2: Z = y @ (Wv.T Wo.T)  (weights pre-fused on host) ----
                for dj in range(ND):
                    tr = pstr2.tile([128, 64], BF16, tag="tr")
                    nc.tensor.transpose(
                        tr[:], y_bf[:, dj * 128:(dj + 1) * 128], idb[0:64, 0:64]
                    )
                    nc.vector.tensor_copy(yt_sb[:, dj, :], tr[:])
                    # keep the PE clock gate warm through the transpose stretch
                    warm2 = pstr2.tile([K, 128], F32, name=f"warm2_{dj}", tag="tr")
                    nc.tensor.matmul(
                        warm2[:], idb[0:64, 0:64], idb[0:64, :],
                        start=True, stop=True,
                    )
                ps_z = [
                    pswz.tile([K, 512], F32, name=f"z{fh}", tag="wz")
                    for fh in range(2)
                ]
                for dj in range(ND):
                    for fh in range(2):
                        nc.tensor.matmul(
                            ps_z[fh][:],
                            yt_sb[:, dj, :],
                            wc_sb[:, dj, fh * 512:(fh + 1) * 512],
                            start=(dj == 0), stop=(dj == ND - 1),
                        )
                for fh in range(2):
                    nc.vector.tensor_copy(
                        z_bf[:, fh * 512:(fh + 1) * 512], ps_z[fh][:]
                    )

                # ---- phase 3: out = aff @ Z (own token half) ----------------------
                for g in range(NOCH // 2):
                    o_sb = o_pool.tile([128, 2, D], BF16)
                    for j2 in range(2):
                        j = g * 2 + j2
                        psum_o = pso.tile([128, D], F32)
                        for fh in range(2):
                            nc.tensor.matmul(
                                psum_o[:, fh * 512:(fh + 1) * 512],
                                afft_sb[:, j * 128:(j + 1) * 128],
                                z_bf[:, fh * 512:(fh + 1) * 512],
                                start=True, stop=True,
                            )
                        # alternate the PSUM->SBUF cast between DVE and ACT so
                        # neither engine paces phase 3
                        if j2 == 0:
                            nc.vector.tensor_copy(o_sb[:, j2, :], psum_o[:])
                        else:
                            nc.scalar.activation(
                                o_sb[:, j2, :], psum_o[:],
                                mybir.ActivationFunctionType.Copy,
                            )
                    dma_eng = nc.sync if g % 2 == 0 else nc.gpsimd
                    dma_eng.dma_start(
                        out_d.ap()[:, g * 2 * D:(g + 1) * 2 * D], o_sb[:]
                    )

    nc.compile()
    return nc


def _get_nc():
    if "nc" not in _CACHE:
        _CACHE["nc"] = _build_nc()
    return _CACHE["nc"]


def kernel(token_embeddings, splat_centers, splat_log_scales, Wv, Wo):
    x = np.asarray(token_embeddings, dtype=np.float32)
    centers = np.asarray(splat_centers, dtype=np.float32)
    log_scales = np.asarray(splat_log_scales, dtype=np.float32)
    Wv = np.asarray(Wv, dtype=np.float32)
    Wo = np.asarray(Wo, dtype=np.float32)

    nc = _get_nc()

    # parameter preprocessing (folded exactly as at model-load time)
    sig = np.clip(np.exp(log_scales), 0.1, 2.0).astype(np.float32)
    inv = (0.5 / (sig * sig)).astype(np.float32)            # 1/(2 sig^2)
    c2 = np.einsum("kd,kd->k", centers, centers).astype(np.float32)

    # constants: identity, fp8 centers (DoubleRow pair layout), f32 affine blob
    cb = np.eye(128, dtype=NPBF16)
    ctb = np.ascontiguousarray((2.0 * CS) * centers.T).astype(NPFP8)   # [D, K]
    # d = djp*256 + i*128 + p  ->  c8[p, djp, i, k]
    c8 = ctb.reshape(NDP, 2, 128, K).transpose(2, 0, 1, 3).reshape(128, -1)
    cf = np.zeros((128, 66), dtype=np.float32)
    cf[:, 0:64] = np.tile(inv.reshape(1, K), (128, 1))
    cf[0:64, 64] = (XS * CS) * c2
    cf[0:64, 65] = inv / (XS * CS)

    wc = (Wv.T.astype(np.float32) @ Wo.T.astype(np.float32)).astype(NPBF16)
    wcr = wc.reshape(ND, 128, D).transpose(1, 0, 2).reshape(128, -1)

    shared = {"cb": cb, "cf": cf, "c8": np.ascontiguousarray(c8),
              "wc": np.ascontiguousarray(wcr)}
    in_maps = []
    for b in range(B):
        xb_bf = x[b].astype(NPBF16)                              # [S, D]
        xbt_f8 = np.ascontiguousarray(x[b].T * XS).astype(NPFP8)  # [D, S]
        for h in range(2):
            own = slice(h * SH, (h + 1) * SH)
            oth = slice((1 - h) * SH, (2 - h) * SH)
            m = dict(shared)
            xn = np.concatenate([xb_bf[own], xb_bf[oth]], axis=0)     # [S, D]
            xt = np.concatenate([xbt_f8[:, own], xbt_f8[:, oth]], axis=1)  # [D, S]
            # xn[blk*512 + c*128 + p, d] -> [p, blk, c, d]
            m["xn"] = np.ascontiguousarray(
                xn.reshape(NB, 4, 128, D).transpose(2, 0, 1, 3).reshape(128, -1)
            )
            # xt[djp*256 + i*128 + p, blk*512 + s] -> [p, blk, djp, i, s]
            m["xt"] = np.ascontiguousarray(
                xt.reshape(NDP, 2, 128, NB, 512).transpose(2, 3, 0, 1, 4)
                .reshape(128, -1)
            )
            in_maps.append(m)

    res = bass_utils.run_bass_kernel_spmd(nc, in_maps, core_ids=list(range(NCORES)))

    out = np.empty((B, S, D), dtype=np.float32)
    for c in range(NCORES):
        b, h = divmod(c, 2)
        # out_lin [128, g, j2, d] -> tokens t = g*256 + j2*128 + p
        ol = res.results[c]["out"].reshape(128, NOCH // 2, 2, D)
        out[b, h * SH:(h + 1) * SH] = (
            ol.transpose(1, 2, 0, 3).reshape(SH, D).astype(np.float32)
        )
    return out
